# revision 22
# baseline (speedup 1.0000x reference)
"""Trainium2 Bass kernel for nn_DecoderGRU (batch-parallel GRU decoder + CE loss).

Self-contained: hardcodes shapes. kernel(**inputs) takes full inputs, shards
batch across 8 NeuronCores (pure data parallel), runs the Bass program via
run_bass_kernel_spmd, gathers full outputs.

Model (matches the jax reference):
  x_t = [emb[char[:, t]], phon]            # [B, 114]
  r = sigmoid(x_t @ W_rx + h @ W_rh + b)   # etc for z
  c = tanh(x_t @ W_hx + (r*h) @ W_hh + b)
  h = (1-z)*h + z*c
  logits[:, t] = h @ W_proj + b_proj
  loss = masked-mean NLL (ignore target==0)

Device layout (per core, B_c=512, 2 column-chunks of 256):
  X1 [128, 512] sbuf: rows 0-63 h (transposed), 64-95 onehot(char), 96-127 phonT[0:32]
  X2 [32, 512]: phonT[32:50] + zero pad
  gates [128, 512]: rows 0-63 r, 64-127 z        (sigmoid out)
  rh [64, 512] = r*h ; c [64,512] = tanh out ; d [64,512] scratch
  All gate GEMMs stream X1/X2/rh chunks (N=256) with weights stacked so that
  emb-table gather reduces to a [32]-onehot matmul (E = emb @ W_emb-part
  premultiplied on host).
"""

import os
import numpy as np
from contextlib import ExitStack

import concourse.bass as bass
import concourse.mybir as mybir
import concourse.tile as tile
from concourse import bacc
from concourse.bass import ts, ds
from concourse.bass_utils import run_bass_kernel_spmd

F32 = mybir.dt.float32
F32R = mybir.dt.float32r
I32 = mybir.dt.int32
I16 = mybir.dt.int16
AF = mybir.ActivationFunctionType
OP = mybir.AluOpType

B, T1, H, P, V = 4096, 65, 64, 50, 32
D = H + P
N_CORES = 8
BC = B // N_CORES          # 512 batch rows per core
T = T1 - 1                 # 64 scan steps
CH = int(os.environ.get("BASS_CH", "256"))  # batch columns per chunk
NCH = BC // CH             # 2 chunks
PBLK = 8                   # proj/loss block length (steps per PSUM flush)
NBLK = T // PBLK           # 8 blocks
NBANK = 2                  # proj psum banks (each holds 2 b-subchunks of 128)

_CACHE = {}


def _f32r(ap):
    return ap if ap.dtype == F32R else ap.bitcast(F32R)


def build_program(t_steps=T, trace_sim=False):
    """Build + compile the per-core Bass program (SPMD: same program, 8 cores)."""
    nc = bacc.Bacc(
        "TRN2", target_bir_lowering=False, debug=False, num_devices=N_CORES
    )

    def din(name, shape, dt=F32):
        return nc.dram_tensor(name, list(shape), dt, kind="ExternalInput").ap()

    def dout(name, shape, dt=F32):
        return nc.dram_tensor(name, list(shape), dt, kind="ExternalOutput").ap()

    # --- DRAM inputs (per core) ---
    d_l1 = din("l1", (128, 128), F32R)        # [Wrzh(64); P_rz(50); 0(14)]
    d_loh = din("loh", (32, 128), F32R)       # E_rz (onehot->rz)
    d_l3 = din("l3", (64, 64), F32R)          # [P_h(50); 0(14)] (phon->h)
    d_l3oh = din("l3oh", (32, 64), F32R)      # E_h (onehot->h)
    d_l4b = din("l4b", (64, 64), F32R)        # W_hh
    d_wproj = din("wp", (64, 32), F32R)       # W_proj
    d_ident = din("id64", (64, 64), F32R)     # identity (z partition-move)
    d_brz = din("brz", (128, 1))
    d_bh = din("bh", (64, 1))
    d_bz = din("bz", (64, 1))
    d_bproj = din("bpj", (128, 512))    # b_proj tiled to logits block layout
    d_phon = din("phn", (64, BC), F32R)       # phonT(50) + zero pad(14)
    d_oh = din("oh", (32, T * BC), F32R)      # onehot stream, [v, t*BC+b]
    d_h0 = din("h0", (64, BC), F32R)    # zeros (fp32r-typed h init)
    d_iota = din("iotav", (128, 2, PBLK, V), I16)  # v-index, v=0 -> -1
    d_ones = din("ones", (128, 1))      # ones column (partition reduce)
    d_tgt = din("tgt", (128, NBANK, 2, T), I16)  # targets [p, bank, sub, t]

    d_logits = dout("logits", (BC, T, V))
    d_loss = dout("lossp", (1, 2))

    with tile.TileContext(nc, trace_sim=trace_sim) as tc, ExitStack() as ctx:
        pool = ctx.enter_context(tc.tile_pool(name="persist", bufs=1))
        lpool = ctx.enter_context(tc.tile_pool(name="logits", bufs=2))
        epool = ctx.enter_context(tc.tile_pool(name="expsc", bufs=2))
        psum = ctx.enter_context(
            tc.tile_pool(name="psum", bufs=1, space=bass.MemorySpace.PSUM)
        )

        # --- persistent SBUF tiles ---
        X1 = pool.tile([128, BC], F32R)          # h(0:64) | phon(64:114) | pad
        ohall = pool.tile([32, T * BC], F32R)    # onehot stream
        gates = pool.tile([128, BC], F32R)       # r | z
        rh = pool.tile([64, BC], F32R)
        zsb = pool.tile([64, BC], F32)
        a_t = pool.tile([64, BC], F32)
        c_t = pool.tile([64, BC], F32)
        d_t = pool.tile([64, BC], F32)
        l1 = pool.tile([128, 128], F32R)
        loh = pool.tile([32, 128], F32R)
        l3 = pool.tile([128, 64], F32R)          # data at partitions 64:128
        l3oh = pool.tile([32, 64], F32R)
        l4b = pool.tile([64, 64], F32R)
        wproj = pool.tile([64, 32], F32R)
        id64 = pool.tile([128, 64], F32R)        # data at partitions 64:128
        brz = pool.tile([128, 1], F32)
        bh = pool.tile([64, 1], F32)
        bz = pool.tile([64, 1], F32)
        bproj = pool.tile([128, 512], F32)
        ones = pool.tile([128, 1], F32)
        tgt = pool.tile([128, NBANK, 2, T], I16)
        iota = pool.tile([128, 2, PBLK, V], I16)  # v-index (v=0 poisoned)
        sumexp = pool.tile([128, NBANK, 2, T], F32)
        acc_tgt = pool.tile([128, NBLK * NBANK], F32)
        acc_lse = pool.tile([128, NBANK], F32)
        acc_cnt = pool.tile([128, NBANK], F32)
        nllcnt = pool.tile([128, 2], F32)
        fin = pool.tile([1, 2], F32)

        # --- PSUM tiles ---
        ps_rz = [psum.tile([128, CH], F32, name=f"ps_rz{i}") for i in range(NCH)]
        ps_h = [psum.tile([64, CH], F32, name=f"ps_h{i}") for i in range(NCH)]
        ps_z = [psum.tile([64, CH], F32, name=f"ps_z{i}") for i in range(NCH)]
        ps_pj = [psum.tile([128, 512], F32, name=f"ps_pj{i}") for i in range(NBANK)]

        dma = nc.sync

        # --- init loads ---
        dma.dma_start(l1[:], d_l1)
        dma.dma_start(loh[:], d_loh)
        dma.dma_start(l3[64:128, :], d_l3)
        dma.dma_start(l3oh[:], d_l3oh)
        dma.dma_start(l4b[:], d_l4b)
        dma.dma_start(wproj[:], d_wproj)
        dma.dma_start(id64[64:128, :], d_ident)
        dma.dma_start(brz[:], d_brz)
        dma.dma_start(bh[:], d_bh)
        dma.dma_start(bz[:], d_bz)
        dma.dma_start(bproj[:], d_bproj)
        dma.dma_start(ones[:], d_ones)
        dma.dma_start(X1[64:128, :], d_phon)
        dma.dma_start(tgt[:], d_tgt)
        dma.dma_start(iota[:], d_iota)
        dma.dma_start(X1[0:64, :], d_h0)        # h0 = 0
        # onehot stream arrives in PBLK-sized slices so step 0 starts early
        for blk in range(NBLK):
            sl = ds(blk * PBLK * BC, PBLK * BC)
            dma.dma_start(ohall[:, sl], d_oh[:, sl])

        for t in range(t_steps):
            blk, tin = t // PBLK, t % PBLK
            for ch in range(NCH):
                cs = ts(ch, CH)
                ohs = ds(t * BC + ch * CH, CH)
                # gate pre-activations: onehot(emb) first (h-independent,
                # prefetchable), then recurrent+phon last on the chain.
                nc.tensor.matmul(
                    ps_rz[ch][:], _f32r(loh[:]), _f32r(ohall[:, ohs]),
                    start=True, stop=False,
                )
                nc.tensor.matmul(
                    ps_rz[ch][:], _f32r(l1[:]), _f32r(X1[:, cs]),
                    start=False, stop=True,
                )
                nc.scalar.activation(
                    gates[:, cs], ps_rz[ch][:], AF.Sigmoid, bias=brz[:]
                )
                # r*h (r rows 0:64 aligned with h rows 0:64)
                nc.vector.scalar_tensor_tensor(
                    rh[:, cs], gates[0:64, cs], 1.0, X1[0:64, cs],
                    op0=OP.mult, op1=OP.mult,
                )
                # z-gate recomputed at partitions 0:64 (weight col-slices),
                # so the update ops read z from SBUF at 2x DVE rate
                nc.tensor.matmul(
                    ps_z[ch][:], _f32r(loh[:, 64:128]), _f32r(ohall[:, ohs]),
                    start=True, stop=False,
                )
                nc.tensor.matmul(
                    ps_z[ch][:], _f32r(l1[:, 64:128]), _f32r(X1[:, cs]),
                    start=False, stop=True,
                )
                nc.scalar.activation(
                    zsb[:, cs], ps_z[ch][:], AF.Sigmoid, bias=bz[:]
                )
                # h-gate: emb(onehot), phon, recurrent
                nc.tensor.matmul(
                    ps_h[ch][:], _f32r(l3oh[:]), _f32r(ohall[:, ohs]),
                    start=True, stop=False,
                )
                nc.tensor.matmul(
                    ps_h[ch][:], _f32r(l3[64:128, :]), _f32r(X1[64:128, cs]),
                    start=False, stop=False,
                )
                nc.tensor.matmul(
                    ps_h[ch][:], _f32r(l4b[:]), _f32r(rh[:, cs]),
                    start=False, stop=True,
                )
                # off-chain (parallel with M3/M4b/tanh): a = z*h ; hma = h - a
                nc.vector.scalar_tensor_tensor(
                    a_t[:, cs], zsb[:, cs], 1.0, X1[0:64, cs],
                    op0=OP.mult, op1=OP.mult,
                )
                nc.vector.scalar_tensor_tensor(
                    d_t[:, cs], X1[0:64, cs], 1.0, a_t[:, cs],
                    op0=OP.mult, op1=OP.subtract,
                )
                nc.scalar.activation(c_t[:, cs], ps_h[ch][:], AF.Tanh, bias=bh[:])
                # on-chain tail: c *= z ; h = hma + z*c
                nc.vector.scalar_tensor_tensor(
                    c_t[:, cs], zsb[:, cs], 1.0, c_t[:, cs],
                    op0=OP.mult, op1=OP.mult,
                )
                nc.vector.scalar_tensor_tensor(
                    X1[0:64, cs], d_t[:, cs], 1.0, c_t[:, cs],
                    op0=OP.mult, op1=OP.add,
                )
                # projection for this step: logits[b, t] = h_new @ W_proj
                for p in range(CH // 128):
                    c128 = ch * (CH // 128) + p
                    bank, half = c128 // 2, c128 % 2
                    nc.tensor.matmul(
                        ps_pj[bank][:, ds(half * 256 + tin * 32, 32)],
                        _f32r(X1[0:64, ds(ch * CH + p * 128, 128)]),
                        _f32r(wproj[:]),
                        start=True, stop=True,
                    )

            flushes = [blk] if tin == PBLK - 1 else []
            for fblk in flushes:
                for bank in range(NBANK):
                    lsb = lpool.tile([128, 512], F32, tag="lsb")
                    esb = epool.tile([128, 512], F32, tag="esb")
                    msb = epool.tile([128, 512], mybir.dt.bfloat16, tag="msb")
                    # logits = psum + b_proj
                    nc.vector.scalar_tensor_tensor(
                        lsb[:], bproj[:], 1.0, ps_pj[bank][:],
                        op0=OP.mult, op1=OP.add,
                    )
                    lsb_v = lsb.rearrange("p (s t v) -> p s t v", s=2, t=PBLK)
                    for sub in range(2):
                        nc.sync.dma_start(
                            d_logits[
                                bank * 256 + sub * 128 : bank * 256 + sub * 128 + 128,
                                fblk * PBLK : (fblk + 1) * PBLK,
                                :,
                            ],
                            lsb_v[:, sub, :, :],
                        )
                    # softmax pieces for the loss
                    nc.scalar.activation(esb[:], lsb[:], AF.Exp)
                    nc.vector.tensor_reduce(
                        sumexp[:, bank, :, fblk * PBLK : (fblk + 1) * PBLK],
                        esb.rearrange("p (s t v) -> p s t v", s=2, t=PBLK),
                        axis=mybir.AxisListType.X,
                        op=OP.add,
                    )
                    tgt_b = tgt[:, bank, :, fblk * PBLK : (fblk + 1) * PBLK]
                    nc.vector.tensor_tensor(
                        msb.rearrange("p (s t v) -> p s t v", s=2, t=PBLK),
                        iota[:],
                        tgt_b.broadcast_to([128, 2, PBLK, V]),
                        op=OP.is_equal,
                    )
                    nc.vector.scalar_tensor_tensor(
                        esb[:],
                        lsb[:],
                        1.0,
                        msb[:],
                        op0=OP.mult,
                        op1=OP.mult,
                        accum_out=acc_tgt[:, fblk * NBANK + bank : fblk * NBANK + bank + 1],
                    )

        # --- loss tail ---
        if t_steps == T:
            for bank in range(NBANK):
                lnl = lpool.tile([128, 2 * T], F32, tag="lnl")
                m01 = lpool.tile([128, 2 * T], F32, tag="m01")
                se_v = sumexp[:, bank, :, :].rearrange("p s t -> p (s t)")
                tg_v = tgt[:, bank, :, :].rearrange("p s t -> p (s t)")
                nc.scalar.activation(lnl[:], se_v, AF.Ln)
                nc.vector.tensor_scalar(
                    m01[:], tg_v, 0, None, op0=OP.not_equal
                )
                nc.vector.scalar_tensor_tensor(
                    lnl[:],
                    lnl[:],
                    1.0,
                    m01[:],
                    op0=OP.mult,
                    op1=OP.mult,
                    accum_out=acc_lse[:, bank : bank + 1],
                )
                nc.vector.tensor_reduce(
                    acc_cnt[:, bank : bank + 1], m01[:],
                    axis=mybir.AxisListType.X, op=OP.add,
                )
            red_t = lpool.tile([128, 1], F32, tag="redt")
            red_l = lpool.tile([128, 1], F32, tag="redl")
            red_c = lpool.tile([128, 1], F32, tag="redc")
            nc.vector.tensor_reduce(
                red_t[:], acc_tgt[:], axis=mybir.AxisListType.X, op=OP.add
            )
            nc.vector.tensor_reduce(
                red_l[:], acc_lse[:], axis=mybir.AxisListType.X, op=OP.add
            )
            nc.vector.tensor_reduce(
                red_c[:], acc_cnt[:], axis=mybir.AxisListType.X, op=OP.add
            )
            # nll_sum = sum(mask*lse) - sum(mask*logit_tgt); reduce over
            # partitions via a [128,1].T @ [128,2] ones-matmul on PE.
            nc.vector.tensor_tensor(
                nllcnt[:, 0:1], red_l[:], red_t[:], op=OP.subtract
            )
            nc.vector.tensor_copy(nllcnt[:, 1:2], red_c[:])
            nc.tensor.matmul(
                ps_z[0][0:1, 0:2], ones[:], nllcnt[:], start=True, stop=True
            )
            nc.vector.tensor_copy(fin[:], ps_z[0][0:1, 0:2])
            nc.sync.dma_start(d_loss, fin[:])

    nc.compile()
    return nc


def prep_inputs(inputs):
    """Host-side: preprocess weights (tiny) + shard batch inputs per core."""
    f = lambda x: np.asarray(x, dtype=np.float32)
    emb = f(inputs["emb"])
    W_rx, W_zx, W_hx = f(inputs["W_rx"]), f(inputs["W_zx"]), f(inputs["W_hx"])
    W_rh, W_zh, W_hh = f(inputs["W_rh"]), f(inputs["W_zh"]), f(inputs["W_hh"])
    W_proj = f(inputs["W_proj"])
    b_rz = np.concatenate(
        [f(inputs["b_rx"]) + f(inputs["b_rh"]), f(inputs["b_zx"]) + f(inputs["b_zh"])]
    )
    b_h = f(inputs["b_hx"]) + f(inputs["b_hh"])
    b_proj = f(inputs["b_proj"])

    E_rz = emb @ np.hstack([W_rx[:H], W_zx[:H]])        # [32, 128]
    E_h = emb @ W_hx[:H]                                # [32, 64]
    P_rz = np.hstack([W_rx[H:], W_zx[H:]])              # [50, 128]
    P_h = W_hx[H:]                                      # [50, 64]
    Wrzh = np.hstack([W_rh, W_zh])                      # [64, 128]

    l1 = np.zeros((128, 128), np.float32)
    l1[0:64] = Wrzh
    l1[64:114] = P_rz
    l3 = np.zeros((64, 64), np.float32)
    l3[0:50] = P_h

    iota = np.broadcast_to(
        np.arange(V, dtype=np.int16), (128, 2, PBLK, V)
    ).copy()
    iota[:, :, :, 0] = -1  # poison v=0 -> implements ignore_index

    shared = {
        "l1": l1,
        "loh": E_rz.astype(np.float32),
        "l3": l3,
        "l3oh": E_h.astype(np.float32),
        "l4b": W_hh.astype(np.float32),
        "wp": W_proj.astype(np.float32),
        "id64": np.eye(64, dtype=np.float32),
        "brz": b_rz.reshape(128, 1).astype(np.float32),
        "bh": b_h.reshape(64, 1).astype(np.float32),
        "bz": b_rz[64:].reshape(64, 1).astype(np.float32),
        "bpj": np.tile(b_proj, (128, 16)).astype(np.float32),
        "iotav": iota,
        "ones": np.ones((128, 1), np.float32),
    }

    phon = f(inputs["phonetic_input"])                  # [B, 50]
    char = np.asarray(inputs["char_seq"]).astype(np.int32)  # [B, 65]

    # onehot over the full batch once: [B, T, V] -> per-core [V, T*BC]
    onehot_full = (
        char[:, :T, None] == np.arange(V, dtype=np.int32)
    ).astype(np.float32)                               # [B, T, V]

    in_maps = []
    for core in range(N_CORES):
        b0 = core * BC
        ph = phon[b0 : b0 + BC].T                       # [50, 512]
        phn = np.zeros((64, BC), np.float32)
        phn[:50] = ph
        tg = char[b0 : b0 + BC, 1:]                     # [512, 64] targets
        oh = np.ascontiguousarray(
            onehot_full[b0 : b0 + BC].transpose(2, 1, 0).reshape(V, T * BC)
        )  # [v, t*BC + b]
        tgt = (
            tg.reshape(NBANK, 2, 128, T).transpose(2, 0, 1, 3).astype(np.int16)
        )  # [p, bank, sub, t]
        m = dict(shared)
        m.update({"phn": phn, "oh": oh, "tgt": tgt,
                  "h0": np.zeros((64, BC), np.float32)})
        in_maps.append(m)
    return in_maps


def kernel(**inputs):
    if "nc" not in _CACHE:
        _CACHE["nc"] = build_program()
    nc = _CACHE["nc"]
    in_maps = prep_inputs(inputs)
    trace = bool(int(os.environ.get("BASS_KERNEL_TRACE", "0")))
    res = run_bass_kernel_spmd(
        nc, in_maps, core_ids=list(range(N_CORES)), trace=trace
    )
    _CACHE["last_result"] = res
    logits = np.concatenate([r["logits"] for r in res.results], axis=0)
    lp = np.stack([r["lossp"] for r in res.results])    # [8, 1, 2]
    nll_sum = lp[:, 0, 0].sum()
    cnt = lp[:, 0, 1].sum()
    loss = np.float32(nll_sum / max(cnt, 1.0))
    return logits.reshape(B, T, V).astype(np.float32), loss
